# revision 16
# baseline (speedup 1.0000x reference)
"""Causal MHA (B=4, S=2048, D=1024, H=16) on 8 NeuronCores.

Sharding: tensor-parallel over heads — 2 heads per core. Each core computes
Q/K/V projections for its 2 heads over the whole batch, causal flash-style
attention, and its partial output projection; the host sums the 8 partials.

Precision: fp32r (E8M11, 1 cycle/row on the PE) everywhere on the
score path; no fp8 residual terms (measured end-to-end rel err 0.009
vs the 2e-2 gate). V, probs, and the output projection run in
bf16/fp16. The causal mask is added by the PE itself (identity
stationary x triangular-FMIN moving matmul) instead of DVE adds, and
the softmax is flash-style: each 512-col score block exps with its
block-local max right after its matmul (releasing the PSUM bank
early); the per-block correction exp(m_b - m)/l folds into the
existing per-block probability scale.
"""

import numpy as np

B, S, D = 4, 2048, 1024
H, E = 16, 64
NCORES = 8
HPC = H // NCORES       # heads per core = 2
E2 = HPC * E            # 128
P = 128
KB = 512                # score block columns (k per psum tile)
SBLK = 512              # s-block for Q/K projection rhs tiles
NDT = D // P            # 8 d-tiles
NQT = S // P            # 16 q-tiles per batch
NST = S // P            # 16 s-tiles per batch
FMIN = -3.0e38

_BUILT = None


def round_fp32r(a):
    """Round fp32 array to E8M11 (round-to-nearest-even on 11 mantissa
    bits) — matches the device DVE fp32->fp32r conversion bit-exactly."""
    u = np.ascontiguousarray(a, np.float32).view(np.uint32)
    keep = np.uint32(0xFFFFF000)
    half = np.uint32(0x800 - 1)
    lsb = (u >> np.uint32(12)) & np.uint32(1)
    r = (u + half + lsb) & keep
    return r.view(np.float32)


def _apply_drain_patch():
    """This walrus build rejects >1 sync-wait on a CTRL (Drain) instruction.
    Split the TileContext exit-drain waits across several drain instructions."""
    import concourse.tile as tile
    import concourse.mybir as mybir
    from concourse.vector_clock import ScopedClock

    if getattr(tile.TileContext, "_drain_patch_applied", False):
        return

    def _drain_and_barrier(self, tick_clock, wait_clock):
        nc = self.nc
        drain_inst = nc.sync.drain()
        wait_clock.add_sem_waits(
            drain_inst.ins, ScopedClock({None: tick_clock.global_clock})
        )
        si = drain_inst.ins.sync_info
        if si is not None and len(si.on_wait) > 1:
            waits = list(si.on_wait)
            del si.on_wait[1:]
            for w in waits[1:]:
                d2 = nc.sync.drain()
                d2.ins.sync_info = mybir.SyncInfo(on_wait=[w], on_update=[])
        nc.all_engine_barrier()
        popped = nc._tile_sem_poison_stack.pop()
        assert popped is self._sem_poison
        nc.clear_and_free_semaphores(list(self.sems.allocated().values()))
        nc.all_engine_barrier()

    tile.TileContext._drain_and_barrier = _drain_and_barrier
    tile.TileContext._drain_patch_applied = True


def _split_multiwaits(nc, max_waits=1):
    """This walrus build rejects instructions carrying more than ~1 sync-wait.
    Hoist extra waits onto single-wait NoOps on the same engine, placed just
    before the instruction (sequencers execute in order, so semantics hold)."""
    import concourse.mybir as mybir

    n_new = 0
    for f in nc.m.functions:
        for blk in f.blocks:
            insts = list(blk.instructions)
            if not any(
                getattr(i, "sync_info", None) is not None
                and len(i.sync_info.on_wait) > max_waits
                for i in insts
            ):
                continue
            out = []
            for inst in insts:
                si = getattr(inst, "sync_info", None)
                if si is not None and len(si.on_wait) > max_waits:
                    extra = list(si.on_wait[max_waits:])
                    del si.on_wait[max_waits:]
                    for w in extra:
                        n_new += 1
                        out.append(mybir.InstNoOp(
                            name=f"splitw-{n_new}",
                            sync_info=mybir.SyncInfo(on_wait=[w], on_update=[]),
                            engine=inst.engine,
                            bass_nofuse=True,
                        ))
                out.append(inst)
            blk.instructions[:] = out
    return n_new


def build_nc(reps=1):
    import concourse.bass as bass
    import concourse.mybir as mybir
    import concourse.tile as tile

    _apply_drain_patch()
    dt = mybir.dt
    Exp = mybir.ActivationFunctionType.Exp
    Copy = mybir.ActivationFunctionType.Copy
    DR = mybir.MatmulPerfMode.DoubleRow

    nc = bass.Bass("TRN2", target_bir_lowering=False, debug=False)

    x_d = nc.dram_tensor("xr", [B, D, S], dt.float32r, kind="ExternalInput").ap()
    qwr_d = nc.dram_tensor("qwr", [D, E2], dt.float32r, kind="ExternalInput").ap()
    kwr_d = nc.dram_tensor("kwr", [D, E2], dt.float32r, kind="ExternalInput").ap()
    vwr_d = nc.dram_tensor("vwr", [D, E2], dt.float32r, kind="ExternalInput").ap()
    ow_d = nc.dram_tensor("ow", [E2, D], dt.bfloat16, kind="ExternalInput").ap()
    id_d = nc.dram_tensor("ident", [P, P], dt.bfloat16, kind="ExternalInput").ap()
    mk_d = nc.dram_tensor("mask", [P, P], dt.bfloat16, kind="ExternalInput").ap()
    out_d = nc.dram_tensor("out", [B, S, D], dt.float16, kind="ExternalOutput").ap()

    with tile.TileContext(nc) as tc:
        with (
            tc.tile_pool(name="const", bufs=1) as cpool,
            tc.tile_pool(name="big", bufs=2) as big,
            tc.tile_pool(name="xs", bufs=2) as xs,
            tc.tile_pool(name="vt", bufs=2) as vt_p,
            tc.tile_pool(name="prow", bufs=5) as prow_p,
            tc.tile_pool(name="pt", bufs=2) as pt_p,
            tc.tile_pool(name="attnT", bufs=2) as attnT_p,
            tc.tile_pool(name="small", bufs=4) as small,
            tc.tile_pool(name="ost", bufs=2) as ost_p,
            # PSUM budget (8 banks): qkv 1 + ss 6 (scores + oproj) + attnT 1
            tc.tile_pool(name="ps1", bufs=2, space="PSUM") as ps1,
            tc.tile_pool(name="psS", bufs=5, space="PSUM") as psS,
            tc.tile_pool(name="psa", bufs=1, space="PSUM") as psa_p,
        ):
            # ---- constants / weights ----
            qwr_w = cpool.tile([P, NDT, E2], dt.float32r, tag="qwr")
            nc.sync.dma_start(qwr_w[:], qwr_d.rearrange("(t p) e -> p t e", p=P))
            kwr_w = cpool.tile([P, NDT, E2], dt.float32r, tag="kwr")
            nc.sync.dma_start(kwr_w[:], kwr_d.rearrange("(t p) e -> p t e", p=P))
            vwr_w = cpool.tile([P, NDT, E2], dt.float32r, tag="vwr")
            nc.sync.dma_start(vwr_w[:], vwr_d.rearrange("(t p) e -> p t e", p=P))
            ow_w = cpool.tile([P, D], dt.bfloat16, tag="ow")
            nc.sync.dma_start(ow_w[:], ow_d)
            id_w = cpool.tile([P, P], dt.bfloat16, tag="idw")
            nc.sync.dma_start(id_w[:], id_d)
            mk_w = cpool.tile([P, P], dt.bfloat16, tag="mkw")
            nc.sync.dma_start(mk_w[:], mk_d)

            # ---- per-batch: projections, then attention, then o_proj ----
            for _rep, b in __import__('itertools').product(range(reps), range(B)):
                QTr = big.tile([P, S], dt.float32r, tag="QT")
                KTr = big.tile([P, S], dt.float32r, tag="KT")
                Vs = big.tile([P, NST, E2], dt.bfloat16, tag="Vs")
                for sb in range(S // SBLK):
                    ssl = slice(sb * SBLK, (sb + 1) * SBLK)
                    xr_t = xs.tile([P, NDT, SBLK], dt.float32r, tag="x")
                    # halved loads so probs transposes can interleave sooner in
                    # the DMA stream (big copies block the xbar-transpose path)
                    xsrc = x_d[b, :, ssl].rearrange("(t p) s -> p t s", p=P)
                    for t in range(0, NDT, 4):
                        nc.scalar.dma_start(xr_t[:, t:t + 4], xsrc[:, t:t + 4])
                    for dst, wr_w in ((QTr, qwr_w), (KTr, kwr_w)):
                        ps = ps1.tile([P, SBLK], dt.float32, tag="qko", name="qk")
                        for t in range(NDT):
                            nc.tensor.matmul(
                                ps[:], wr_w[:, t], xr_t[:, t],
                                start=(t == 0), stop=(t == NDT - 1),
                            )
                        nc.vector.tensor_copy(out=dst[:, ssl], in_=ps[:])
                    # V chunk from the fp32r x (no residual needed), then
                    # DMA-transpose into Vs
                    psv = ps1.tile([P, SBLK], dt.float32, tag="qko", name="v")
                    for t in range(NDT):
                        nc.tensor.matmul(
                            psv[:], vwr_w[:, t], xr_t[:, t],
                            start=(t == 0), stop=(t == NDT - 1),
                        )
                    vt_t = vt_p.tile([P, SBLK], dt.bfloat16, tag="vt")
                    nc.vector.tensor_copy(out=vt_t[:], in_=psv[:])
                    nc.sync.dma_start_transpose(
                        out=Vs[:, sb * (SBLK // P):(sb + 1) * (SBLK // P), :],
                        in_=vt_t[:],
                    )

                # ---- attention for this batch ----
                attnT_sb = attnT_p.tile([P, NST, E2], dt.bfloat16, tag="attnT")

                def oproj_st(st):
                    for dhalf in range(2):
                        pso = psS.tile([P, 512], dt.float32, tag="ss", name="o")
                        nc.tensor.matmul(
                            pso[:], attnT_sb[:, st, :],
                            ow_w[:, dhalf * 512:(dhalf + 1) * 512],
                            start=True, stop=True,
                        )
                        osb = ost_p.tile([P, 512], dt.float16, tag="ost")
                        nc.scalar.copy(out=osb[:], in_=pso[:])
                        nc.sync.dma_start(
                            out_d[b, st * P:(st + 1) * P,
                                  dhalf * 512:(dhalf + 1) * 512],
                            osb[:],
                        )

                def emit_pv_col(g, pt, psa, j):
                    # PV for q-column j of group g (q-tile 4g+j): causal sum
                    # over k-tiles 0..qt only. One PSUM accumulation group per
                    # head spans all 4 columns of the bank: start only at
                    # (j==0, kt==0); the per-element has_written bits make
                    # each column's first matmul overwrite. Probs are already
                    # normalized so this directly yields attnT.
                    qt = 4 * g + j
                    for kt in range(qt + 1):
                        for h in range(HPC):
                            nc.tensor.matmul(
                                psa[h * E:(h + 1) * E, j * P:(j + 1) * P],
                                Vs[:, kt, h * E:(h + 1) * E],
                                pt[h][:, kt, j * P:(j + 1) * P],
                                start=(j == 0 and kt == 0),
                                stop=(j == 3 and kt == qt),
                            )
                    if j == 3:
                        nc.vector.tensor_copy(
                            out=attnT_sb[:, 4 * g:4 * (g + 1), :]
                                .rearrange("p a b -> p (a b)"),
                            in_=psa[:],
                        )

                pending = None
                for g in range(NQT // 4):
                    kext = g + 1  # causal extent of the whole group, in KB blocks
                    pt = [pt_p.tile([P, NST, 4 * P], dt.bfloat16, tag=f"pt{h}",
                                    name=f"pt{h}") for h in range(HPC)]
                    for j in range(4):
                        qt = 4 * g + j
                        # valid columns in the last (diagonal) block
                        vext = (j + 1) * P
                        for h in range(HPC):
                            hs = slice(h * E, (h + 1) * E)
                            qsl = slice(qt * P, (qt + 1) * P)
                            nmb = small.tile([P, 4], dt.float32, tag=f"nmb{h}",
                                             name=f"nmb{h}")
                            lbuf = small.tile([P, 4], dt.float32, tag=f"lb{h}",
                                              name=f"lb{h}")
                            prow = prow_p.tile([P, S], dt.bfloat16,
                                               tag=f"prow{h}", name=f"prow{h}")
                            # flash-style: per-block local max + exp, so each
                            # PSUM bank is released right after its exp
                            # instead of at the end of the whole q-row
                            for kb in range(kext):
                                diag = kb == kext - 1
                                nv = vext if diag else KB
                                # fp32r matmuls need >=256 moving rows for
                                # full rate; extra columns are masked
                                nvc = max(nv, 256)
                                ksl = slice(kb * KB, kb * KB + nvc)
                                pss = psS.tile([P, KB], dt.float32, tag="ss")
                                nc.tensor.matmul(
                                    pss[:, :nvc], QTr[hs, qsl], KTr[hs, ksl],
                                    start=True, stop=not diag,
                                )
                                if diag:
                                    # causal mask added on the PE: ident^T @
                                    # triangular-FMIN lands on the last 128
                                    # valid columns
                                    nc.tensor.matmul(
                                        pss[:, nv - P:nv], id_w[:], mk_w[:],
                                        start=False, stop=True,
                                    )
                                nc.vector.reduce_max(
                                    out=nmb[:, kb:kb + 1], in_=pss[:, :nv],
                                    axis=mybir.AxisListType.X, negate=True,
                                )
                                nc.scalar.activation(
                                    out=prow[:, kb * KB:kb * KB + nv],
                                    in_=pss[:, :nv], func=Exp,
                                    bias=nmb[:, kb:kb + 1], scale=1.0,
                                    accum_out=lbuf[:, kb:kb + 1],
                                )
                            # finalize: combine block maxes/sums, then scale
                            # each block by c_kb = exp(m_kb - m) / l
                            lr_h = small.tile([P, 1], dt.float32, tag=f"lr{h}",
                                              name=f"lr{h}")
                            if kext == 1:
                                nc.vector.reciprocal(lr_h[:], lbuf[:, 0:1])
                                nc.vector.tensor_scalar_mul(
                                    prow[:, :(qt + 1) * P],
                                    prow[:, :(qt + 1) * P], lr_h[:])
                            else:
                                negm = small.tile([P, 1], dt.float32,
                                                  tag=f"negm{h}",
                                                  name=f"negm{h}")
                                nc.vector.tensor_reduce(
                                    out=negm[:], in_=nmb[:, :kext],
                                    op=mybir.AluOpType.min,
                                    axis=mybir.AxisListType.X,
                                )
                                dpre = small.tile([P, 4], dt.float32,
                                                  tag=f"dp{h}", name=f"dp{h}")
                                nc.vector.tensor_scalar_sub(
                                    dpre[:, :kext], nmb[:, :kext], negm[:])
                                dd = small.tile([P, 4], dt.float32,
                                                tag=f"dd{h}", name=f"dd{h}")
                                nc.scalar.activation(
                                    out=dd[:, :kext], in_=dpre[:, :kext],
                                    func=Exp, scale=-1.0)
                                dl = small.tile([P, 4], dt.float32,
                                                tag=f"dl{h}", name=f"dl{h}")
                                nc.vector.tensor_tensor(
                                    dl[:, :kext], dd[:, :kext],
                                    lbuf[:, :kext], mybir.AluOpType.mult)
                                l_h = small.tile([P, 1], dt.float32,
                                                 tag=f"l{h}", name=f"l{h}")
                                nc.vector.reduce_sum(
                                    out=l_h[:], in_=dl[:, :kext],
                                    axis=mybir.AxisListType.X,
                                )
                                nc.vector.reciprocal(lr_h[:], l_h[:])
                                cc = small.tile([P, 4], dt.float32,
                                                tag=f"cc{h}", name=f"cc{h}")
                                nc.vector.tensor_scalar_mul(
                                    cc[:, :kext], dd[:, :kext], lr_h[:])
                                for kb in range(kext):
                                    nv = vext if kb == kext - 1 else KB
                                    nc.vector.tensor_scalar_mul(
                                        prow[:, kb * KB:kb * KB + nv],
                                        prow[:, kb * KB:kb * KB + nv],
                                        cc[:, kb:kb + 1])
                            nc.sync.dma_start_transpose(
                                out=pt[h][:, :qt + 1, j * P:(j + 1) * P],
                                in_=prow[:, :(qt + 1) * P],
                            )
                        if pending is not None:
                            emit_pv_col(*pending, j)
                    pending = (g, pt, psa_p.tile([P, 4 * P], dt.float32,
                                                 tag="a", name="a"))
                if pending is not None:
                    g_l, pt_l, psa_l = pending
                    for j in range(4):
                        emit_pv_col(g_l, pt_l, psa_l, j)
                    pending = None
                # ---- phase C: partial output projection for this batch ----
                for st in range(NST):
                    oproj_st(st)

    _split_multiwaits(nc)
    return nc


def make_in_maps(in_feature, q_proj, k_proj, v_proj, o_proj):
    import ml_dtypes

    bf16 = ml_dtypes.bfloat16
    x = np.asarray(in_feature, np.float32)
    xT = np.ascontiguousarray(x.transpose(0, 2, 1))          # [B, D, S]
    xr = round_fp32r(xT)

    scale = np.float32(1.0 / np.sqrt(E))
    qw = np.asarray(q_proj, np.float32).reshape(H, E, D) * scale
    kw = np.asarray(k_proj, np.float32).reshape(H, E, D)
    vw = np.asarray(v_proj, np.float32).reshape(H, E, D)
    ow = np.asarray(o_proj, np.float32).reshape(D, H, E)

    ident = np.eye(P, dtype=np.float32).astype(bf16)
    mask = np.where(np.arange(P)[None, :] <= np.arange(P)[:, None],
                    0.0, FMIN).astype(bf16)

    in_maps = []
    for c in range(NCORES):
        sl = slice(HPC * c, HPC * (c + 1))
        qT = np.ascontiguousarray(qw[sl].reshape(E2, D).T)   # [D, E2]
        kT = np.ascontiguousarray(kw[sl].reshape(E2, D).T)
        vT = np.ascontiguousarray(vw[sl].reshape(E2, D).T)
        oT = np.ascontiguousarray(ow[:, sl, :].reshape(D, E2).T)  # [E2, D]
        in_maps.append({
            "xr": xr,
            "qwr": round_fp32r(qT), "kwr": round_fp32r(kT),
            "vwr": round_fp32r(vT),
            "ow": oT.astype(bf16), "ident": ident, "mask": mask,
        })
    return in_maps


def kernel(in_feature, q_proj, k_proj, v_proj, o_proj, _results_hook=None):
    from concourse.bass_utils import run_bass_kernel_spmd

    global _BUILT
    if _BUILT is None:
        _BUILT = build_nc()
    in_maps = make_in_maps(in_feature, q_proj, k_proj, v_proj, o_proj)
    res = run_bass_kernel_spmd(_BUILT, in_maps, core_ids=list(range(NCORES)))
    if _results_hook is not None:
        _results_hook(res)
    out = np.zeros((B, S, D), np.float32)
    for r in res.results:
        out += np.asarray(r["out"], np.float32)
    return out



# revision 17
# speedup vs baseline: 1.0973x; 1.0973x over previous
"""Causal MHA (B=4, S=2048, D=1024, H=16) on 8 NeuronCores.

Sharding: tensor-parallel over heads — 2 heads per core. Each core computes
Q/K/V projections for its 2 heads over the whole batch, causal flash-style
attention, and its partial output projection; the host sums the 8 partials.

Precision: fp32r (E8M11, 1 cycle/row on the PE) everywhere on the
score path; no fp8 residual terms (measured end-to-end rel err 0.009
vs the 2e-2 gate). V, probs, and the output projection run in
bf16/fp16. The causal mask is added by the PE itself (identity
stationary x triangular-FMIN moving matmul) instead of DVE adds, and
the softmax is flash-style: each 512-col score block exps with its
block-local max right after its matmul (releasing the PSUM bank
early); the per-block correction exp(m_b - m)/l folds into the
existing per-block probability scale.
"""

import numpy as np

B, S, D = 4, 2048, 1024
H, E = 16, 64
NCORES = 8
HPC = H // NCORES       # heads per core = 2
E2 = HPC * E            # 128
P = 128
KB = 512                # score block columns (k per psum tile)
SBLK = 512              # s-block for Q/K projection rhs tiles
NDT = D // P            # 8 d-tiles
NQT = S // P            # 16 q-tiles per batch
NST = S // P            # 16 s-tiles per batch
FMIN = -3.0e38

_BUILT = None


def round_fp32r(a):
    """Round fp32 array to E8M11 (round-to-nearest-even on 11 mantissa
    bits) — matches the device DVE fp32->fp32r conversion bit-exactly."""
    u = np.ascontiguousarray(a, np.float32).view(np.uint32)
    keep = np.uint32(0xFFFFF000)
    half = np.uint32(0x800 - 1)
    lsb = (u >> np.uint32(12)) & np.uint32(1)
    r = (u + half + lsb) & keep
    return r.view(np.float32)


def _apply_drain_patch():
    """This walrus build rejects >1 sync-wait on a CTRL (Drain) instruction.
    Split the TileContext exit-drain waits across several drain instructions."""
    import concourse.tile as tile
    import concourse.mybir as mybir
    from concourse.vector_clock import ScopedClock

    if getattr(tile.TileContext, "_drain_patch_applied", False):
        return

    def _drain_and_barrier(self, tick_clock, wait_clock):
        nc = self.nc
        drain_inst = nc.sync.drain()
        wait_clock.add_sem_waits(
            drain_inst.ins, ScopedClock({None: tick_clock.global_clock})
        )
        si = drain_inst.ins.sync_info
        if si is not None and len(si.on_wait) > 1:
            waits = list(si.on_wait)
            del si.on_wait[1:]
            for w in waits[1:]:
                d2 = nc.sync.drain()
                d2.ins.sync_info = mybir.SyncInfo(on_wait=[w], on_update=[])
        nc.all_engine_barrier()
        popped = nc._tile_sem_poison_stack.pop()
        assert popped is self._sem_poison
        nc.clear_and_free_semaphores(list(self.sems.allocated().values()))
        nc.all_engine_barrier()

    tile.TileContext._drain_and_barrier = _drain_and_barrier
    tile.TileContext._drain_patch_applied = True


def _split_multiwaits(nc, max_waits=1):
    """This walrus build rejects instructions carrying more than ~1 sync-wait.
    Hoist extra waits onto single-wait NoOps on the same engine, placed just
    before the instruction (sequencers execute in order, so semantics hold)."""
    import concourse.mybir as mybir

    n_new = 0
    for f in nc.m.functions:
        for blk in f.blocks:
            insts = list(blk.instructions)
            if not any(
                getattr(i, "sync_info", None) is not None
                and len(i.sync_info.on_wait) > max_waits
                for i in insts
            ):
                continue
            out = []
            for inst in insts:
                si = getattr(inst, "sync_info", None)
                if si is not None and len(si.on_wait) > max_waits:
                    extra = list(si.on_wait[max_waits:])
                    del si.on_wait[max_waits:]
                    for w in extra:
                        n_new += 1
                        out.append(mybir.InstNoOp(
                            name=f"splitw-{n_new}",
                            sync_info=mybir.SyncInfo(on_wait=[w], on_update=[]),
                            engine=inst.engine,
                            bass_nofuse=True,
                        ))
                out.append(inst)
            blk.instructions[:] = out
    return n_new


def build_nc(reps=1):
    import concourse.bass as bass
    import concourse.mybir as mybir
    import concourse.tile as tile

    _apply_drain_patch()
    dt = mybir.dt
    Exp = mybir.ActivationFunctionType.Exp
    Copy = mybir.ActivationFunctionType.Copy
    DR = mybir.MatmulPerfMode.DoubleRow

    nc = bass.Bass("TRN2", target_bir_lowering=False, debug=False)

    x_d = nc.dram_tensor("xr", [B, D, S], dt.float32r, kind="ExternalInput").ap()
    qwr_d = nc.dram_tensor("qwr", [D, E2], dt.float32r, kind="ExternalInput").ap()
    kwr_d = nc.dram_tensor("kwr", [D, E2], dt.float32r, kind="ExternalInput").ap()
    vwr_d = nc.dram_tensor("vwr", [D, E2], dt.float32r, kind="ExternalInput").ap()
    ow_d = nc.dram_tensor("ow", [E2, D], dt.bfloat16, kind="ExternalInput").ap()
    id_d = nc.dram_tensor("ident", [P, P], dt.bfloat16, kind="ExternalInput").ap()
    mk_d = nc.dram_tensor("mask", [P, P], dt.bfloat16, kind="ExternalInput").ap()
    out_d = nc.dram_tensor("out", [B, S, D], dt.float16, kind="ExternalOutput").ap()

    with tile.TileContext(nc) as tc:
        with (
            tc.tile_pool(name="const", bufs=1) as cpool,
            tc.tile_pool(name="big", bufs=2) as big,
            tc.tile_pool(name="xs", bufs=2) as xs,
            tc.tile_pool(name="vt", bufs=2) as vt_p,
            tc.tile_pool(name="prow", bufs=5) as prow_p,
            tc.tile_pool(name="pt", bufs=1) as pt_p,
            tc.tile_pool(name="attnT", bufs=2) as attnT_p,
            tc.tile_pool(name="small", bufs=8) as small,
            tc.tile_pool(name="ost", bufs=4) as ost_p,
            # PSUM budget (8 banks): qkv 1 + ss 6 (scores + oproj) + attnT 1
            tc.tile_pool(name="ps1", bufs=2, space="PSUM") as ps1,
            tc.tile_pool(name="psS", bufs=5, space="PSUM") as psS,
            tc.tile_pool(name="psa", bufs=1, space="PSUM") as psa_p,
        ):
            # ---- constants / weights ----
            qwr_w = cpool.tile([P, NDT, E2], dt.float32r, tag="qwr")
            nc.sync.dma_start(qwr_w[:], qwr_d.rearrange("(t p) e -> p t e", p=P))
            kwr_w = cpool.tile([P, NDT, E2], dt.float32r, tag="kwr")
            nc.sync.dma_start(kwr_w[:], kwr_d.rearrange("(t p) e -> p t e", p=P))
            vwr_w = cpool.tile([P, NDT, E2], dt.float32r, tag="vwr")
            nc.sync.dma_start(vwr_w[:], vwr_d.rearrange("(t p) e -> p t e", p=P))
            ow_w = cpool.tile([P, D], dt.bfloat16, tag="ow")
            nc.sync.dma_start(ow_w[:], ow_d)
            id_w = cpool.tile([P, P], dt.bfloat16, tag="idw")
            nc.sync.dma_start(id_w[:], id_d)
            mk_w = cpool.tile([P, P], dt.bfloat16, tag="mkw")
            nc.sync.dma_start(mk_w[:], mk_d)

            # ---- per-batch: projections, then attention, then o_proj ----
            for _rep, b in __import__('itertools').product(range(reps), range(B)):
                QTr = big.tile([P, S], dt.float32r, tag="QT")
                KTr = big.tile([P, S], dt.float32r, tag="KT")
                Vs = big.tile([P, NST, E2], dt.bfloat16, tag="Vs")
                for sb in range(S // SBLK):
                    ssl = slice(sb * SBLK, (sb + 1) * SBLK)
                    xr_t = xs.tile([P, NDT, SBLK], dt.float32r, tag="x")
                    # halved loads so probs transposes can interleave sooner in
                    # the DMA stream (big copies block the xbar-transpose path)
                    xsrc = x_d[b, :, ssl].rearrange("(t p) s -> p t s", p=P)
                    for t in range(0, NDT, 4):
                        nc.scalar.dma_start(xr_t[:, t:t + 4], xsrc[:, t:t + 4])
                    for dst, wr_w in ((QTr, qwr_w), (KTr, kwr_w)):
                        ps = ps1.tile([P, SBLK], dt.float32, tag="qko", name="qk")
                        for t in range(NDT):
                            nc.tensor.matmul(
                                ps[:], wr_w[:, t], xr_t[:, t],
                                start=(t == 0), stop=(t == NDT - 1),
                            )
                        nc.vector.tensor_copy(out=dst[:, ssl], in_=ps[:])
                    # V chunk from the fp32r x (no residual needed), then
                    # DMA-transpose into Vs
                    psv = ps1.tile([P, SBLK], dt.float32, tag="qko", name="v")
                    for t in range(NDT):
                        nc.tensor.matmul(
                            psv[:], vwr_w[:, t], xr_t[:, t],
                            start=(t == 0), stop=(t == NDT - 1),
                        )
                    vt_t = vt_p.tile([P, SBLK], dt.bfloat16, tag="vt")
                    nc.vector.tensor_copy(out=vt_t[:], in_=psv[:])
                    nc.sync.dma_start_transpose(
                        out=Vs[:, sb * (SBLK // P):(sb + 1) * (SBLK // P), :],
                        in_=vt_t[:],
                    )

                # ---- attention for this batch ----
                attnT_sb = attnT_p.tile([P, NST, E2], dt.bfloat16, tag="attnT")

                def oproj_st(st):
                    for dhalf in range(2):
                        pso = psS.tile([P, 512], dt.float32, tag="ss", name="o")
                        nc.tensor.matmul(
                            pso[:], attnT_sb[:, st, :],
                            ow_w[:, dhalf * 512:(dhalf + 1) * 512],
                            start=True, stop=True,
                        )
                        osb = ost_p.tile([P, 512], dt.float16, tag="ost")
                        nc.scalar.copy(out=osb[:], in_=pso[:])
                        nc.sync.dma_start(
                            out_d[b, st * P:(st + 1) * P,
                                  dhalf * 512:(dhalf + 1) * 512],
                            osb[:],
                        )

                def emit_pv_col(g, pt, psa, j):
                    # PV for q-column j of group g (q-tile 4g+j): causal sum
                    # over k-tiles 0..qt only. One PSUM accumulation group per
                    # head spans all 4 columns of the bank: start only at
                    # (j==0, kt==0); the per-element has_written bits make
                    # each column's first matmul overwrite. Probs are already
                    # normalized so this directly yields attnT.
                    qt = 4 * g + j
                    for kt in range(qt + 1):
                        for h in range(HPC):
                            nc.tensor.matmul(
                                psa[h * E:(h + 1) * E, j * P:(j + 1) * P],
                                Vs[:, kt, h * E:(h + 1) * E],
                                pt[h][:, kt, j * P:(j + 1) * P],
                                start=(j == 0 and kt == 0),
                                stop=(j == 3 and kt == qt),
                            )
                    if j == 3:
                        nc.vector.tensor_copy(
                            out=attnT_sb[:, 4 * g:4 * (g + 1), :]
                                .rearrange("p a b -> p (a b)"),
                            in_=psa[:],
                        )

                pending = None
                for g in range(NQT // 4):
                    kext = g + 1  # causal extent of the whole group, in KB blocks
                    pt = [pt_p.tile([P, NST, 4 * P], dt.bfloat16, tag=f"pt{h}",
                                    name=f"pt{h}") for h in range(HPC)]
                    for j in range(4):
                        qt = 4 * g + j
                        # valid columns in the last (diagonal) block
                        vext = (j + 1) * P
                        for h in range(HPC):
                            hs = slice(h * E, (h + 1) * E)
                            qsl = slice(qt * P, (qt + 1) * P)
                            nmb = small.tile([P, 4], dt.float32, tag=f"nmb{h}",
                                             name=f"nmb{h}")
                            lbuf = small.tile([P, 4], dt.float32, tag=f"lb{h}",
                                              name=f"lb{h}")
                            prow = prow_p.tile([P, S], dt.bfloat16,
                                               tag=f"prow{h}", name=f"prow{h}")
                            # flash-style: per-block local max + exp, so each
                            # PSUM bank is released right after its exp
                            # instead of at the end of the whole q-row
                            for kb in range(kext):
                                diag = kb == kext - 1
                                nv = vext if diag else KB
                                # fp32r matmuls need >=256 moving rows for
                                # full rate; extra columns are masked
                                nvc = max(nv, 256)
                                ksl = slice(kb * KB, kb * KB + nvc)
                                pss = psS.tile([P, KB], dt.float32, tag="ss")
                                nc.tensor.matmul(
                                    pss[:, :nvc], QTr[hs, qsl], KTr[hs, ksl],
                                    start=True, stop=not diag,
                                )
                                if diag:
                                    # causal mask added on the PE: ident^T @
                                    # triangular-FMIN lands on the last 128
                                    # valid columns
                                    nc.tensor.matmul(
                                        pss[:, nv - P:nv], id_w[:], mk_w[:],
                                        start=False, stop=True,
                                    )
                                nc.vector.reduce_max(
                                    out=nmb[:, kb:kb + 1], in_=pss[:, :nv],
                                    axis=mybir.AxisListType.X, negate=True,
                                )
                                nc.scalar.activation(
                                    out=prow[:, kb * KB:kb * KB + nv],
                                    in_=pss[:, :nv], func=Exp,
                                    bias=nmb[:, kb:kb + 1], scale=1.0,
                                    accum_out=lbuf[:, kb:kb + 1],
                                )
                            # finalize: combine block maxes/sums, then scale
                            # each block by c_kb = exp(m_kb - m) / l
                            lr_h = small.tile([P, 1], dt.float32, tag=f"lr{h}",
                                              name=f"lr{h}")
                            if kext == 1:
                                nc.vector.reciprocal(lr_h[:], lbuf[:, 0:1])
                                nc.vector.tensor_scalar_mul(
                                    prow[:, :(qt + 1) * P],
                                    prow[:, :(qt + 1) * P], lr_h[:])
                            else:
                                negm = small.tile([P, 1], dt.float32,
                                                  tag=f"negm{h}",
                                                  name=f"negm{h}")
                                nc.vector.tensor_reduce(
                                    out=negm[:], in_=nmb[:, :kext],
                                    op=mybir.AluOpType.min,
                                    axis=mybir.AxisListType.X,
                                )
                                dpre = small.tile([P, 4], dt.float32,
                                                  tag=f"dp{h}", name=f"dp{h}")
                                nc.vector.tensor_scalar_sub(
                                    dpre[:, :kext], nmb[:, :kext], negm[:])
                                dd = small.tile([P, 4], dt.float32,
                                                tag=f"dd{h}", name=f"dd{h}")
                                nc.scalar.activation(
                                    out=dd[:, :kext], in_=dpre[:, :kext],
                                    func=Exp, scale=-1.0)
                                dl = small.tile([P, 4], dt.float32,
                                                tag=f"dl{h}", name=f"dl{h}")
                                nc.vector.tensor_tensor(
                                    dl[:, :kext], dd[:, :kext],
                                    lbuf[:, :kext], mybir.AluOpType.mult)
                                l_h = small.tile([P, 1], dt.float32,
                                                 tag=f"l{h}", name=f"l{h}")
                                nc.vector.reduce_sum(
                                    out=l_h[:], in_=dl[:, :kext],
                                    axis=mybir.AxisListType.X,
                                )
                                nc.vector.reciprocal(lr_h[:], l_h[:])
                                cc = small.tile([P, 4], dt.float32,
                                                tag=f"cc{h}", name=f"cc{h}")
                                nc.vector.tensor_scalar_mul(
                                    cc[:, :kext], dd[:, :kext], lr_h[:])
                                for kb in range(kext):
                                    nv = vext if kb == kext - 1 else KB
                                    nc.vector.tensor_scalar_mul(
                                        prow[:, kb * KB:kb * KB + nv],
                                        prow[:, kb * KB:kb * KB + nv],
                                        cc[:, kb:kb + 1])
                            nc.sync.dma_start_transpose(
                                out=pt[h][:, :qt + 1, j * P:(j + 1) * P],
                                in_=prow[:, :(qt + 1) * P],
                            )
                        if pending is not None:
                            emit_pv_col(*pending, j)
                    pending = (g, pt, psa_p.tile([P, 4 * P], dt.float32,
                                                 tag="a", name="a"))
                if pending is not None:
                    g_l, pt_l, psa_l = pending
                    for j in range(4):
                        emit_pv_col(g_l, pt_l, psa_l, j)
                    pending = None
                # ---- phase C: partial output projection for this batch ----
                for st in range(NST):
                    oproj_st(st)

    _split_multiwaits(nc)
    return nc


def make_in_maps(in_feature, q_proj, k_proj, v_proj, o_proj):
    import ml_dtypes

    bf16 = ml_dtypes.bfloat16
    x = np.asarray(in_feature, np.float32)
    xT = np.ascontiguousarray(x.transpose(0, 2, 1))          # [B, D, S]
    xr = round_fp32r(xT)

    scale = np.float32(1.0 / np.sqrt(E))
    qw = np.asarray(q_proj, np.float32).reshape(H, E, D) * scale
    kw = np.asarray(k_proj, np.float32).reshape(H, E, D)
    vw = np.asarray(v_proj, np.float32).reshape(H, E, D)
    ow = np.asarray(o_proj, np.float32).reshape(D, H, E)

    ident = np.eye(P, dtype=np.float32).astype(bf16)
    mask = np.where(np.arange(P)[None, :] <= np.arange(P)[:, None],
                    0.0, FMIN).astype(bf16)

    in_maps = []
    for c in range(NCORES):
        sl = slice(HPC * c, HPC * (c + 1))
        qT = np.ascontiguousarray(qw[sl].reshape(E2, D).T)   # [D, E2]
        kT = np.ascontiguousarray(kw[sl].reshape(E2, D).T)
        vT = np.ascontiguousarray(vw[sl].reshape(E2, D).T)
        oT = np.ascontiguousarray(ow[:, sl, :].reshape(D, E2).T)  # [E2, D]
        in_maps.append({
            "xr": xr,
            "qwr": round_fp32r(qT), "kwr": round_fp32r(kT),
            "vwr": round_fp32r(vT),
            "ow": oT.astype(bf16), "ident": ident, "mask": mask,
        })
    return in_maps


def kernel(in_feature, q_proj, k_proj, v_proj, o_proj, _results_hook=None):
    from concourse.bass_utils import run_bass_kernel_spmd

    global _BUILT
    if _BUILT is None:
        _BUILT = build_nc()
    in_maps = make_in_maps(in_feature, q_proj, k_proj, v_proj, o_proj)
    res = run_bass_kernel_spmd(_BUILT, in_maps, core_ids=list(range(NCORES)))
    if _results_hook is not None:
        _results_hook(res)
    out = np.zeros((B, S, D), np.float32)
    for r in res.results:
        out += np.asarray(r["out"], np.float32)
    return out



# revision 18
# speedup vs baseline: 1.1113x; 1.0127x over previous
"""Causal MHA (B=4, S=2048, D=1024, H=16) on 8 NeuronCores.

Sharding: tensor-parallel over heads — 2 heads per core. Each core computes
Q/K/V projections for its 2 heads over the whole batch, causal flash-style
attention, and its partial output projection; the host sums the 8 partials.

Precision: fp32r (E8M11, 1 cycle/row on the PE) everywhere on the
score path; no fp8 residual terms (measured end-to-end rel err 0.009
vs the 2e-2 gate). V, probs, and the output projection run in
bf16/fp16. The causal mask is added by the PE itself (identity
stationary x triangular-FMIN moving matmul) instead of DVE adds, and
the softmax is flash-style: each 512-col score block exps with its
block-local max right after its matmul (releasing the PSUM bank
early); the per-block correction exp(m_b - m)/l folds into the
existing per-block probability scale.
"""

import numpy as np

B, S, D = 4, 2048, 1024
H, E = 16, 64
NCORES = 8
HPC = H // NCORES       # heads per core = 2
E2 = HPC * E            # 128
P = 128
KB = 512                # score block columns (k per psum tile)
SBLK = 512              # s-block for Q/K projection rhs tiles
NDT = D // P            # 8 d-tiles
NQT = S // P            # 16 q-tiles per batch
NST = S // P            # 16 s-tiles per batch
FMIN = -3.0e38

_BUILT = None


def round_fp32r(a):
    """Round fp32 array to E8M11 (round-to-nearest-even on 11 mantissa
    bits) — matches the device DVE fp32->fp32r conversion bit-exactly."""
    u = np.ascontiguousarray(a, np.float32).view(np.uint32)
    keep = np.uint32(0xFFFFF000)
    half = np.uint32(0x800 - 1)
    lsb = (u >> np.uint32(12)) & np.uint32(1)
    r = (u + half + lsb) & keep
    return r.view(np.float32)


def _apply_drain_patch():
    """This walrus build rejects >1 sync-wait on a CTRL (Drain) instruction.
    Split the TileContext exit-drain waits across several drain instructions."""
    import concourse.tile as tile
    import concourse.mybir as mybir
    from concourse.vector_clock import ScopedClock

    if getattr(tile.TileContext, "_drain_patch_applied", False):
        return

    def _drain_and_barrier(self, tick_clock, wait_clock):
        nc = self.nc
        drain_inst = nc.sync.drain()
        wait_clock.add_sem_waits(
            drain_inst.ins, ScopedClock({None: tick_clock.global_clock})
        )
        si = drain_inst.ins.sync_info
        if si is not None and len(si.on_wait) > 1:
            waits = list(si.on_wait)
            del si.on_wait[1:]
            for w in waits[1:]:
                d2 = nc.sync.drain()
                d2.ins.sync_info = mybir.SyncInfo(on_wait=[w], on_update=[])
        nc.all_engine_barrier()
        popped = nc._tile_sem_poison_stack.pop()
        assert popped is self._sem_poison
        nc.clear_and_free_semaphores(list(self.sems.allocated().values()))
        nc.all_engine_barrier()

    tile.TileContext._drain_and_barrier = _drain_and_barrier
    tile.TileContext._drain_patch_applied = True


def _split_multiwaits(nc, max_waits=1):
    """This walrus build rejects instructions carrying more than ~1 sync-wait.
    Hoist extra waits onto single-wait NoOps on the same engine, placed just
    before the instruction (sequencers execute in order, so semantics hold)."""
    import concourse.mybir as mybir

    n_new = 0
    for f in nc.m.functions:
        for blk in f.blocks:
            insts = list(blk.instructions)
            if not any(
                getattr(i, "sync_info", None) is not None
                and len(i.sync_info.on_wait) > max_waits
                for i in insts
            ):
                continue
            out = []
            for inst in insts:
                si = getattr(inst, "sync_info", None)
                if si is not None and len(si.on_wait) > max_waits:
                    extra = list(si.on_wait[max_waits:])
                    del si.on_wait[max_waits:]
                    for w in extra:
                        n_new += 1
                        out.append(mybir.InstNoOp(
                            name=f"splitw-{n_new}",
                            sync_info=mybir.SyncInfo(on_wait=[w], on_update=[]),
                            engine=inst.engine,
                            bass_nofuse=True,
                        ))
                out.append(inst)
            blk.instructions[:] = out
    return n_new


def build_nc(reps=1):
    import concourse.bass as bass
    import concourse.mybir as mybir
    import concourse.tile as tile

    _apply_drain_patch()
    dt = mybir.dt
    Exp = mybir.ActivationFunctionType.Exp
    Copy = mybir.ActivationFunctionType.Copy
    DR = mybir.MatmulPerfMode.DoubleRow

    nc = bass.Bass("TRN2", target_bir_lowering=False, debug=False)

    x_d = nc.dram_tensor("xr", [B, D, S], dt.float32r, kind="ExternalInput").ap()
    qwr_d = nc.dram_tensor("qwr", [D, E2], dt.float32r, kind="ExternalInput").ap()
    kwr_d = nc.dram_tensor("kwr", [D, E2], dt.float32r, kind="ExternalInput").ap()
    vwr_d = nc.dram_tensor("vwr", [D, E2], dt.float32r, kind="ExternalInput").ap()
    ow_d = nc.dram_tensor("ow", [E2, D], dt.bfloat16, kind="ExternalInput").ap()
    id_d = nc.dram_tensor("ident", [P, P], dt.bfloat16, kind="ExternalInput").ap()
    mk_d = nc.dram_tensor("mask", [P, P], dt.bfloat16, kind="ExternalInput").ap()
    out_d = nc.dram_tensor("out", [B, S, D], dt.float16, kind="ExternalOutput").ap()

    with tile.TileContext(nc) as tc:
        with (
            tc.tile_pool(name="const", bufs=1) as cpool,
            tc.tile_pool(name="big", bufs=2) as big,
            tc.tile_pool(name="xs", bufs=2) as xs,
            tc.tile_pool(name="vt", bufs=2) as vt_p,
            tc.tile_pool(name="prow", bufs=5) as prow_p,
            tc.tile_pool(name="pt", bufs=1) as pt_p,
            tc.tile_pool(name="attnT", bufs=2) as attnT_p,
            tc.tile_pool(name="small", bufs=16) as small,
            tc.tile_pool(name="ost", bufs=6) as ost_p,
            # PSUM budget (8 banks): qkv 1 + ss 6 (scores + oproj) + attnT 1
            tc.tile_pool(name="ps1", bufs=2, space="PSUM") as ps1,
            tc.tile_pool(name="psS", bufs=5, space="PSUM") as psS,
            tc.tile_pool(name="psa", bufs=1, space="PSUM") as psa_p,
        ):
            # ---- constants / weights ----
            qwr_w = cpool.tile([P, NDT, E2], dt.float32r, tag="qwr")
            nc.sync.dma_start(qwr_w[:], qwr_d.rearrange("(t p) e -> p t e", p=P))
            kwr_w = cpool.tile([P, NDT, E2], dt.float32r, tag="kwr")
            nc.sync.dma_start(kwr_w[:], kwr_d.rearrange("(t p) e -> p t e", p=P))
            vwr_w = cpool.tile([P, NDT, E2], dt.float32r, tag="vwr")
            nc.sync.dma_start(vwr_w[:], vwr_d.rearrange("(t p) e -> p t e", p=P))
            ow_w = cpool.tile([P, D], dt.bfloat16, tag="ow")
            nc.sync.dma_start(ow_w[:], ow_d)
            id_w = cpool.tile([P, P], dt.bfloat16, tag="idw")
            nc.sync.dma_start(id_w[:], id_d)
            mk_w = cpool.tile([P, P], dt.bfloat16, tag="mkw")
            nc.sync.dma_start(mk_w[:], mk_d)

            # ---- per-batch: projections, then attention, then o_proj ----
            for _rep, b in __import__('itertools').product(range(reps), range(B)):
                QTr = big.tile([P, S], dt.float32r, tag="QT")
                KTr = big.tile([P, S], dt.float32r, tag="KT")
                Vs = big.tile([P, NST, E2], dt.bfloat16, tag="Vs")
                for sb in range(S // SBLK):
                    ssl = slice(sb * SBLK, (sb + 1) * SBLK)
                    xr_t = xs.tile([P, NDT, SBLK], dt.float32r, tag="x")
                    # halved loads so probs transposes can interleave sooner in
                    # the DMA stream (big copies block the xbar-transpose path)
                    xsrc = x_d[b, :, ssl].rearrange("(t p) s -> p t s", p=P)
                    for t in range(0, NDT, 4):
                        nc.scalar.dma_start(xr_t[:, t:t + 4], xsrc[:, t:t + 4])
                    for dst, wr_w in ((QTr, qwr_w), (KTr, kwr_w)):
                        ps = ps1.tile([P, SBLK], dt.float32, tag="qko", name="qk")
                        for t in range(NDT):
                            nc.tensor.matmul(
                                ps[:], wr_w[:, t], xr_t[:, t],
                                start=(t == 0), stop=(t == NDT - 1),
                            )
                        nc.vector.tensor_copy(out=dst[:, ssl], in_=ps[:])
                    # V chunk from the fp32r x (no residual needed), then
                    # DMA-transpose into Vs
                    psv = ps1.tile([P, SBLK], dt.float32, tag="qko", name="v")
                    for t in range(NDT):
                        nc.tensor.matmul(
                            psv[:], vwr_w[:, t], xr_t[:, t],
                            start=(t == 0), stop=(t == NDT - 1),
                        )
                    vt_t = vt_p.tile([P, SBLK], dt.bfloat16, tag="vt")
                    nc.vector.tensor_copy(out=vt_t[:], in_=psv[:])
                    nc.sync.dma_start_transpose(
                        out=Vs[:, sb * (SBLK // P):(sb + 1) * (SBLK // P), :],
                        in_=vt_t[:],
                    )

                # ---- attention for this batch ----
                attnT_sb = attnT_p.tile([P, NST, E2], dt.bfloat16, tag="attnT")

                def oproj_st(st):
                    for dhalf in range(2):
                        pso = psS.tile([P, 512], dt.float32, tag="ss", name="o")
                        nc.tensor.matmul(
                            pso[:], attnT_sb[:, st, :],
                            ow_w[:, dhalf * 512:(dhalf + 1) * 512],
                            start=True, stop=True,
                        )
                        osb = ost_p.tile([P, 512], dt.float16, tag="ost")
                        nc.scalar.copy(out=osb[:], in_=pso[:])
                        nc.sync.dma_start(
                            out_d[b, st * P:(st + 1) * P,
                                  dhalf * 512:(dhalf + 1) * 512],
                            osb[:],
                        )

                def emit_pv_col(g, pt, psa, j):
                    # PV for q-column j of group g (q-tile 4g+j): causal sum
                    # over k-tiles 0..qt only. One PSUM accumulation group per
                    # head spans all 4 columns of the bank: start only at
                    # (j==0, kt==0); the per-element has_written bits make
                    # each column's first matmul overwrite. Probs are already
                    # normalized so this directly yields attnT.
                    qt = 4 * g + j
                    for kt in range(qt + 1):
                        for h in range(HPC):
                            nc.tensor.matmul(
                                psa[h * E:(h + 1) * E, j * P:(j + 1) * P],
                                Vs[:, kt, h * E:(h + 1) * E],
                                pt[h][:, kt, j * P:(j + 1) * P],
                                start=(j == 0 and kt == 0),
                                stop=(j == 3 and kt == qt),
                            )
                    if j == 3:
                        nc.vector.tensor_copy(
                            out=attnT_sb[:, 4 * g:4 * (g + 1), :]
                                .rearrange("p a b -> p (a b)"),
                            in_=psa[:],
                        )

                pending = None
                for g in range(NQT // 4):
                    kext = g + 1  # causal extent of the whole group, in KB blocks
                    pt = [pt_p.tile([P, NST, 4 * P], dt.bfloat16, tag=f"pt{h}",
                                    name=f"pt{h}") for h in range(HPC)]
                    for j in range(4):
                        qt = 4 * g + j
                        # valid columns in the last (diagonal) block
                        vext = (j + 1) * P
                        for h in range(HPC):
                            hs = slice(h * E, (h + 1) * E)
                            qsl = slice(qt * P, (qt + 1) * P)
                            nmb = small.tile([P, 4], dt.float32, tag=f"nmb{h}",
                                             name=f"nmb{h}")
                            lbuf = small.tile([P, 4], dt.float32, tag=f"lb{h}",
                                              name=f"lb{h}")
                            prow = prow_p.tile([P, S], dt.bfloat16,
                                               tag=f"prow{h}", name=f"prow{h}")
                            # flash-style: per-block local max + exp, so each
                            # PSUM bank is released right after its exp
                            # instead of at the end of the whole q-row
                            for kb in range(kext):
                                diag = kb == kext - 1
                                nv = vext if diag else KB
                                # fp32r matmuls need >=256 moving rows for
                                # full rate; extra columns are masked
                                nvc = max(nv, 256)
                                ksl = slice(kb * KB, kb * KB + nvc)
                                pss = psS.tile([P, KB], dt.float32, tag="ss")
                                nc.tensor.matmul(
                                    pss[:, :nvc], QTr[hs, qsl], KTr[hs, ksl],
                                    start=True, stop=not diag,
                                )
                                if diag:
                                    # causal mask added on the PE: ident^T @
                                    # triangular-FMIN lands on the last 128
                                    # valid columns
                                    nc.tensor.matmul(
                                        pss[:, nv - P:nv], id_w[:], mk_w[:],
                                        start=False, stop=True,
                                    )
                                nc.vector.reduce_max(
                                    out=nmb[:, kb:kb + 1], in_=pss[:, :nv],
                                    axis=mybir.AxisListType.X, negate=True,
                                )
                                nc.scalar.activation(
                                    out=prow[:, kb * KB:kb * KB + nv],
                                    in_=pss[:, :nv], func=Exp,
                                    bias=nmb[:, kb:kb + 1], scale=1.0,
                                    accum_out=lbuf[:, kb:kb + 1],
                                )
                            # finalize: combine block maxes/sums, then scale
                            # each block by c_kb = exp(m_kb - m) / l
                            lr_h = small.tile([P, 1], dt.float32, tag=f"lr{h}",
                                              name=f"lr{h}")
                            if kext == 1:
                                nc.vector.reciprocal(lr_h[:], lbuf[:, 0:1])
                                nc.vector.tensor_scalar_mul(
                                    prow[:, :(qt + 1) * P],
                                    prow[:, :(qt + 1) * P], lr_h[:])
                            else:
                                negm = small.tile([P, 1], dt.float32,
                                                  tag=f"negm{h}",
                                                  name=f"negm{h}")
                                nc.vector.tensor_reduce(
                                    out=negm[:], in_=nmb[:, :kext],
                                    op=mybir.AluOpType.min,
                                    axis=mybir.AxisListType.X,
                                )
                                dpre = small.tile([P, 4], dt.float32,
                                                  tag=f"dp{h}", name=f"dp{h}")
                                nc.vector.tensor_scalar_sub(
                                    dpre[:, :kext], nmb[:, :kext], negm[:])
                                dd = small.tile([P, 4], dt.float32,
                                                tag=f"dd{h}", name=f"dd{h}")
                                nc.scalar.activation(
                                    out=dd[:, :kext], in_=dpre[:, :kext],
                                    func=Exp, scale=-1.0)
                                dl = small.tile([P, 4], dt.float32,
                                                tag=f"dl{h}", name=f"dl{h}")
                                nc.vector.tensor_tensor(
                                    dl[:, :kext], dd[:, :kext],
                                    lbuf[:, :kext], mybir.AluOpType.mult)
                                l_h = small.tile([P, 1], dt.float32,
                                                 tag=f"l{h}", name=f"l{h}")
                                nc.vector.reduce_sum(
                                    out=l_h[:], in_=dl[:, :kext],
                                    axis=mybir.AxisListType.X,
                                )
                                nc.vector.reciprocal(lr_h[:], l_h[:])
                                cc = small.tile([P, 4], dt.float32,
                                                tag=f"cc{h}", name=f"cc{h}")
                                nc.vector.tensor_scalar_mul(
                                    cc[:, :kext], dd[:, :kext], lr_h[:])
                                for kb in range(kext):
                                    nv = vext if kb == kext - 1 else KB
                                    nc.vector.tensor_scalar_mul(
                                        prow[:, kb * KB:kb * KB + nv],
                                        prow[:, kb * KB:kb * KB + nv],
                                        cc[:, kb:kb + 1])
                            nc.sync.dma_start_transpose(
                                out=pt[h][:, :qt + 1, j * P:(j + 1) * P],
                                in_=prow[:, :(qt + 1) * P],
                            )
                        if pending is not None:
                            emit_pv_col(*pending, j)
                    pending = (g, pt, psa_p.tile([P, 4 * P], dt.float32,
                                                 tag="a", name="a"))
                if pending is not None:
                    g_l, pt_l, psa_l = pending
                    for j in range(4):
                        emit_pv_col(g_l, pt_l, psa_l, j)
                    pending = None
                # ---- phase C: partial output projection for this batch ----
                for st in range(NST):
                    oproj_st(st)

    _split_multiwaits(nc)
    return nc


def make_in_maps(in_feature, q_proj, k_proj, v_proj, o_proj):
    import ml_dtypes

    bf16 = ml_dtypes.bfloat16
    x = np.asarray(in_feature, np.float32)
    xT = np.ascontiguousarray(x.transpose(0, 2, 1))          # [B, D, S]
    xr = round_fp32r(xT)

    scale = np.float32(1.0 / np.sqrt(E))
    qw = np.asarray(q_proj, np.float32).reshape(H, E, D) * scale
    kw = np.asarray(k_proj, np.float32).reshape(H, E, D)
    vw = np.asarray(v_proj, np.float32).reshape(H, E, D)
    ow = np.asarray(o_proj, np.float32).reshape(D, H, E)

    ident = np.eye(P, dtype=np.float32).astype(bf16)
    mask = np.where(np.arange(P)[None, :] <= np.arange(P)[:, None],
                    0.0, FMIN).astype(bf16)

    in_maps = []
    for c in range(NCORES):
        sl = slice(HPC * c, HPC * (c + 1))
        qT = np.ascontiguousarray(qw[sl].reshape(E2, D).T)   # [D, E2]
        kT = np.ascontiguousarray(kw[sl].reshape(E2, D).T)
        vT = np.ascontiguousarray(vw[sl].reshape(E2, D).T)
        oT = np.ascontiguousarray(ow[:, sl, :].reshape(D, E2).T)  # [E2, D]
        in_maps.append({
            "xr": xr,
            "qwr": round_fp32r(qT), "kwr": round_fp32r(kT),
            "vwr": round_fp32r(vT),
            "ow": oT.astype(bf16), "ident": ident, "mask": mask,
        })
    return in_maps


def kernel(in_feature, q_proj, k_proj, v_proj, o_proj, _results_hook=None):
    from concourse.bass_utils import run_bass_kernel_spmd

    global _BUILT
    if _BUILT is None:
        _BUILT = build_nc()
    in_maps = make_in_maps(in_feature, q_proj, k_proj, v_proj, o_proj)
    res = run_bass_kernel_spmd(_BUILT, in_maps, core_ids=list(range(NCORES)))
    if _results_hook is not None:
        _results_hook(res)
    out = np.zeros((B, S, D), np.float32)
    for r in res.results:
        out += np.asarray(r["out"], np.float32)
    return out

